# revision 11
# baseline (speedup 1.0000x reference)
"""MLA + DeepSeekMoE block on 8 trn2 NeuronCores (Bass/Tile SPMD).

Token-sharded across 8 cores (512 tokens each, causally balanced stripes).
Attention uses the MLA absorption trick with the q-side absorption product
(W_qnope @ W_kvnope^T) precomputed on the host, so scores/outputs are
computed against the shared 320-dim KV latent; the only collective is one
AllGather of the normalized latent per batch group of 4 cores.

bf16 matmul path everywhere (fp32r disables PE fast-weight-load). rms2
sumsq + gate logits stay fp32 so the top-2 selection is robust.

Routed experts are SPARSE: per expert the selected tokens (~128 of 512,
capacity 192) are compacted with gpsimd sparse_gather on a (token-if-
selected-else -1) grid, dispatched with ap_gather from a token-major copy
of h2, and combined with gpsimd scatter_add into a zeroed bf16 buffer.
"""

import numpy as np
import ml_dtypes

import concourse.bacc as bacc
import concourse.mybir as mybir
from concourse.tile import TileContext
from concourse.bass_utils import run_bass_kernel_spmd

# ---- problem constants ----
D = 1024; H = 8; QLR = 384; KVLR = 256; NOPE = 128; ROPE = 64; VD = 128
E = 8; TOPK = 2; INTER = 512; NSH = 2; B = 2; T = 2048; QKD = NOPE + ROPE
N = B * T
NCORES = 8
NLOC = N // NCORES          # 512 tokens per core
P = 128
EPS = 1e-6
SCALE = 1.0 / np.sqrt(QKD)
NEG = -1e9
C = 192                     # per-expert token capacity (max real load ~163)
CI = C // 16

F32 = mybir.dt.float32
BF16 = mybir.dt.bfloat16
I16 = mybir.dt.int16
U32 = mybir.dt.uint32


def _kmap(a):
    """abs k-tile (0..15, 128 tokens each) -> (group-local core block 0..3,
    local tile 0..3 within that core's 512-token contribution)."""
    s = a // 2
    blk = s if s < 4 else 7 - s
    ktl = (a % 2) + (0 if s < 4 else 2)
    return blk, ktl


def _rope_perm():
    return np.concatenate([np.arange(0, ROPE, 2), np.arange(1, ROPE, 2)])


def _core_positions(c):
    """batch-local positions (512,) of core c's tokens."""
    j = c % 4
    return np.concatenate([np.arange(j * 256, (j + 1) * 256),
                           np.arange((7 - j) * 256, (8 - j) * 256)])


def _tile_w(w):
    """[K, F] row-major -> [128, K//128, F] partition-major contiguous."""
    K, F = w.shape
    return np.ascontiguousarray(w.reshape(K // P, P, F).transpose(1, 0, 2))


# ============================ device program ============================

def build():
    from contextlib import ExitStack
    nc = bacc.Bacc(name="mla_moe")

    # ---- I/O ----
    xT = nc.dram_tensor("xT", [D, NLOC], F32, kind="ExternalInput")
    xTbf = nc.dram_tensor("xTbf", [D, NLOC], BF16, kind="ExternalInput")
    cosT = nc.dram_tensor("cosT", [ROPE // 2, NLOC], F32, kind="ExternalInput")
    sinT = nc.dram_tensor("sinT", [ROPE // 2, NLOC], F32, kind="ExternalInput")
    mask1 = nc.dram_tensor("mask1", [8, P, NLOC], BF16, kind="ExternalInput")
    mask2 = nc.dram_tensor("mask2", [8, P, 256], BF16, kind="ExternalInput")
    ident = nc.dram_tensor("ident", [P, P], F32, kind="ExternalInput")
    identb = nc.dram_tensor("identb", [P, P], BF16, kind="ExternalInput")
    ones32 = nc.dram_tensor("ones32", [P, 1], F32, kind="ExternalInput")
    onesbf = nc.dram_tensor("onesbf", [P, 1], BF16, kind="ExternalInput")
    iota = nc.dram_tensor("iota", [P, 1], F32, kind="ExternalInput")
    lat_w = nc.dram_tensor("lat_w", [P, 8, QLR + KVLR + ROPE], BF16, kind="ExternalInput")
    q_up = nc.dram_tensor("q_up", [P, 3, H * QKD], BF16, kind="ExternalInput")
    wabs = nc.dram_tensor("wabs", [P, 3, H * KVLR], BF16, kind="ExternalInput")
    wv_w = nc.dram_tensor("wv_w", [P, 2, H * VD], BF16, kind="ExternalInput")
    cproj_w = nc.dram_tensor("cproj_w", [P, 8, D], BF16, kind="ExternalInput")
    gate_w = nc.dram_tensor("gate_w", [P, 8, E], F32, kind="ExternalInput")
    shw1 = nc.dram_tensor("shw1", [P, 8, INTER * NSH], BF16, kind="ExternalInput")
    shw3 = nc.dram_tensor("shw3", [P, 8, INTER * NSH], BF16, kind="ExternalInput")
    shw2 = nc.dram_tensor("shw2", [P, 8, D], BF16, kind="ExternalInput")
    ew1 = nc.dram_tensor("ew1", [E, P, 8, INTER], BF16, kind="ExternalInput")
    ew3 = nc.dram_tensor("ew3", [E, P, 8, INTER], BF16, kind="ExternalInput")
    ew2 = nc.dram_tensor("ew2", [E, P, 4, D], BF16, kind="ExternalInput")
    sel8 = nc.dram_tensor("sel8", [E, E * P], BF16, kind="ExternalInput")
    rep16 = nc.dram_tensor("rep16", [16, P], F32, kind="ExternalInput")
    out_xT = nc.dram_tensor("out_xT", [D, NLOC], F32, kind="ExternalOutput")

    cc_in = nc.dram_tensor("cc_in", [384, NLOC], BF16)
    vald = nc.dram_tensor("vald", [P, E, 4], F32)
    cc_out = nc.dram_tensor("cc_out", [4 * 384, NLOC], BF16)
    RG = [[0, 1, 2, 3], [4, 5, 6, 7]]

    AL = mybir.AluOpType
    AF = mybir.ActivationFunctionType

    with TileContext(nc) as tc, \
         tc.tile_pool(name="const", bufs=1) as p_const, \
         tc.tile_pool(name="rows", bufs=1) as p_rows, \
         tc.tile_pool(name="psc", bufs=2) as p_sc:

        # ---- long-lived pool groups, closed manually at phase boundaries ----
        g_x = ExitStack();    p_x = g_x.enter_context(tc.tile_pool(name="px", bufs=1, side="right"))
        g_y = ExitStack();    p_y = g_y.enter_context(tc.tile_pool(name="py", bufs=1, side="right"))
        g_q = ExitStack()
        g_kv = ExitStack()
        g_mask = ExitStack()

        identf = p_const.tile([P, P], F32, tag="identf")
        nc.sync.dma_start(out=identf[:], in_=ident[:])
        identb_sb = p_const.tile([P, P], BF16, tag="identb")
        nc.sync.dma_start(out=identb_sb[:], in_=identb[:])
        ones32_sb = p_const.tile([P, 1], F32, tag="ones32")
        nc.sync.dma_start(out=ones32_sb[:], in_=ones32[:])
        onesbf_sb = p_const.tile([P, 1], BF16, tag="onesbf")
        nc.sync.dma_start(out=onesbf_sb[:], in_=onesbf[:])
        iota_sb = p_const.tile([P, 1], F32, tag="iota")
        nc.sync.dma_start(out=iota_sb[:], in_=iota[:])
        cos_sb = p_const.tile([ROPE // 2, NLOC], F32, tag="cos")
        nc.sync.dma_start(out=cos_sb[:], in_=cosT[:])
        sin_sb = p_const.tile([ROPE // 2, NLOC], F32, tag="sin")
        nc.sync.dma_start(out=sin_sb[:], in_=sinT[:])
        eps1 = p_const.tile([1, 1], F32, tag="eps1")
        nc.vector.memset(eps1[:], EPS)
        xT_sb = p_x.tile([P, 8, NLOC], BF16, tag="xT")
        nc.sync.dma_start(out=xT_sb[:], in_=xTbf.rearrange("(a p) n -> p a n", p=P))

        rows_sb = p_rows.tile([1, 3, NLOC], F32, tag="rows")

        ones1_sb = p_const.tile([1, P], F32, tag="ones1")
        nc.vector.memset(ones1_sb[:], 1.0)
        sel8_sb = p_const.tile([E, E * P], BF16, tag="sel8")
        nc.sync.dma_start(out=sel8_sb[:], in_=sel8[:])

        def brec(ps_bc, sb_pool, row_ap, n=NLOC, tag="bc"):
            # broadcast a [1, n] row (PE ones-matmul) then reciprocal of the
            # broadcast tile ([128, n] DVE op; [1, n] runs on one partition)
            bc = ps_bc.tile([P, NLOC], F32, tag=tag)
            nc.tensor.matmul(bc[:, :n], ones1_sb[:], row_ap, start=True, stop=True)
            sb = sb_pool.tile([P, NLOC], F32, tag="bcsb")
            nc.vector.reciprocal(out=sb[:, :n], in_=bc[:, :n])
            return sb

        # ================= phase A: latents, norms, rope, gather, q =================
        with tc.tile_pool(name="weq", bufs=1) as p_weq, \
             tc.tile_pool(name="acta", bufs=1) as p_actA, \
             tc.tile_pool(name="qsc", bufs=2) as p_qsc:

            qup_sb = p_weq.tile([P, 3, H * QKD], BF16, tag="qup")
            nc.sync.dma_start(out=qup_sb[:], in_=q_up[:])
            wabs_sb = p_weq.tile([P, 3, H * KVLR], BF16, tag="wabs")
            nc.sync.dma_start(out=wabs_sb[:], in_=wabs[:])
            qln = p_actA.tile([P, 3, NLOC], BF16, tag="qln")
            kvn = p_actA.tile([P, 2, NLOC], BF16, tag="kvn")
            kr = p_actA.tile([ROPE, NLOC], BF16, tag="kr")
            scr = p_actA.tile([ROPE // 2, NLOC], F32, tag="krs")

            with tc.tile_pool(name="welat", bufs=1) as p_welat, \
                 tc.tile_pool(name="plat", bufs=1) as p_lat, \
                 tc.tile_pool(name="pslat", bufs=6, space="PSUM") as ps_lat, \
                 tc.tile_pool(name="psrow", bufs=1, space="PSUM") as ps_row, \
                 tc.tile_pool(name="psbc", bufs=1, space="PSUM") as ps_bc:

                latw_sb = p_welat.tile([P, 8, QLR + KVLR + ROPE], BF16, tag="latw")
                nc.sync.dma_start(out=latw_sb[:], in_=lat_w[:])

                # rms1 sumsq via bf16 squares + ones matmul
                ss_ps = ps_row.tile([1, NLOC], F32, tag="ss")
                for ds in range(8):
                    xsq = p_sc.tile([P, NLOC], BF16, tag="xsq")
                    nc.vector.tensor_mul(out=xsq[:], in0=xT_sb[:, ds, :], in1=xT_sb[:, ds, :])
                    nc.tensor.matmul(ss_ps[:], onesbf_sb[:], xsq[:],
                                     start=(ds == 0), stop=(ds == 7))
                nc.scalar.activation(out=rows_sb[:, 1, :], in_=ss_ps[:],
                                     func=AF.Sqrt, bias=eps1[:], scale=1.0 / D)
                # latent matmul (scale by 1/sqrt on copyback)
                latT = p_lat.tile([P, 6, NLOC], BF16, tag="latT")
                s1b = brec(ps_bc, p_sc, rows_sb[:, 1, :])
                fts = [(0, 128), (128, 128), (256, 128), (384, 128), (512, 128), (640, 64)]
                for ft, (f0, fsz) in enumerate(fts):
                    lp = ps_lat.tile([P, NLOC], F32, tag="lat")
                    for ds in range(8):
                        nc.tensor.matmul(lp[:fsz], latw_sb[:, ds, f0:f0 + fsz],
                                         xT_sb[:, ds, :],
                                         start=(ds == 0), stop=(ds == 7))
                    nc.vector.tensor_tensor(out=latT[:fsz, ft, :], in0=lp[:fsz],
                                            in1=s1b[:fsz], op=AL.mult)

                # q-norm scale (fold attention score scale in)
                sq_ps = ps_row.tile([1, NLOC], F32, tag="ss")
                for t in range(3):
                    xsq = p_sc.tile([P, NLOC], BF16, tag="xsq")
                    nc.vector.tensor_mul(out=xsq[:], in0=latT[:, t, :], in1=latT[:, t, :])
                    nc.tensor.matmul(sq_ps[:], onesbf_sb[:], xsq[:],
                                     start=(t == 0), stop=(t == 2))
                nc.scalar.activation(out=rows_sb[:, 1, :], in_=sq_ps[:],
                                     func=AF.Sqrt, bias=eps1[:], scale=1.0 / QLR)
                sqb = brec(ps_bc, p_sc, rows_sb[:, 1, :])
                nc.vector.tensor_scalar_mul(out=sqb[:], in0=sqb[:], scalar1=float(SCALE))
                for t in range(3):
                    nc.vector.tensor_tensor(out=qln[:, t, :], in0=latT[:, t, :],
                                            in1=sqb[:], op=AL.mult)

                # kv-norm scale
                skv_ps = ps_row.tile([1, NLOC], F32, tag="ss")
                for i, t in enumerate((3, 4)):
                    xsq = p_sc.tile([P, NLOC], BF16, tag="xsq")
                    nc.vector.tensor_mul(out=xsq[:], in0=latT[:, t, :], in1=latT[:, t, :])
                    nc.tensor.matmul(skv_ps[:], onesbf_sb[:], xsq[:],
                                     start=(i == 0), stop=(i == 1))
                nc.scalar.activation(out=rows_sb[:, 1, :], in_=skv_ps[:],
                                     func=AF.Sqrt, bias=eps1[:], scale=1.0 / KVLR)
                skvb = brec(ps_bc, p_sc, rows_sb[:, 1, :])
                for t in range(2):
                    nc.vector.tensor_tensor(out=kvn[:, t, :], in0=latT[:, 3 + t, :],
                                            in1=skvb[:], op=AL.mult)

                # k_r rope (rows deinterleaved by host weight permutation).
                ev = latT[0:32, 5, :]
                odc = p_actA.tile([ROPE // 2, NLOC], F32, tag="odc")
                scr2 = p_actA.tile([ROPE // 2, NLOC], F32, tag="krs2")
                nc.vector.tensor_copy(out=odc[:], in_=latT[32:64, 5, :])
                nc.vector.tensor_mul(out=kr[0:32], in0=ev, in1=cos_sb[:])
                nc.vector.tensor_mul(out=scr[:], in0=odc[:], in1=sin_sb[:])
                nc.vector.tensor_sub(out=kr[0:32], in0=kr[0:32], in1=scr[:])
                nc.vector.tensor_mul(out=scr[:], in0=ev, in1=sin_sb[:])
                nc.vector.tensor_mul(out=scr2[:], in0=odc[:], in1=cos_sb[:])
                nc.vector.tensor_add(out=scr[:], in0=scr[:], in1=scr2[:])
                nc.vector.tensor_copy(out=kr[32:64], in_=scr[:])

            # contribution -> internal DRAM -> AllGather (per batch group)
            nc.sync.dma_start(out=cc_in[0:128], in_=kvn[:, 0, :])
            nc.sync.dma_start(out=cc_in[128:256], in_=kvn[:, 1, :])
            nc.sync.dma_start(out=cc_in[256:320], in_=kr[:])
            nc.sync.dma_start(out=cc_in[320:384], in_=kr[:])  # pad, never read
            nc.gpsimd.collective_compute(
                "AllGather", AL.bypass, ins=[cc_in[:]], outs=[cc_out[:]],
                replica_groups=RG)

            # q side per head (overlaps the AllGather); q_abs via host-absorbed W
            p_q = g_q.enter_context(tc.tile_pool(name="pq", bufs=1, side="right"))
            qabs = p_q.tile([P, 2 * H, NLOC], BF16, tag="qabs")
            qrope = p_q.tile([ROPE, H, NLOC], BF16, tag="qrope")
            with tc.tile_pool(name="psqp", bufs=3, space="PSUM") as ps_qp:
                for h in range(H):
                    for i in range(2):
                        qa_ps = ps_qp.tile([P, NLOC], F32, tag="qp")
                        for t in range(3):
                            nc.tensor.matmul(qa_ps[:],
                                             wabs_sb[:, t, (2 * h + i) * 128:(2 * h + i + 1) * 128],
                                             qln[:, t, :], start=(t == 0), stop=(t == 2))
                        nc.vector.tensor_copy(out=qabs[:, h * 2 + i, :], in_=qa_ps[:])
                    qr_ps = ps_qp.tile([P, NLOC], F32, tag="qp")
                    for t in range(3):
                        nc.tensor.matmul(qr_ps[:ROPE],
                                         qup_sb[:, t, h * QKD + NOPE:(h + 1) * QKD],
                                         qln[:, t, :], start=(t == 0), stop=(t == 2))
                    qsc = p_qsc.tile([ROPE // 2, NLOC], F32, tag="qsc")
                    qsc2 = p_qsc.tile([ROPE // 2, NLOC], F32, tag="qsc2")
                    nc.vector.tensor_mul(out=qrope[0:32, h, :], in0=qr_ps[0:32], in1=cos_sb[:])
                    nc.vector.tensor_mul(out=qsc[:], in0=qr_ps[32:64], in1=sin_sb[:])
                    nc.vector.tensor_sub(out=qrope[0:32, h, :], in0=qrope[0:32, h, :], in1=qsc[:])
                    nc.vector.tensor_mul(out=qsc[:], in0=qr_ps[0:32], in1=sin_sb[:])
                    nc.vector.tensor_mul(out=qsc2[:], in0=qr_ps[32:64], in1=cos_sb[:])
                    nc.vector.tensor_add(out=qsc[:], in0=qsc[:], in1=qsc2[:])
                    nc.vector.tensor_copy(out=qrope[32:64, h, :], in_=qsc[:])

        # ================= gathered KV: fm blocks + tm transposes =================
        p_kv = g_kv.enter_context(tc.tile_pool(name="pkv", bufs=1, side="right"))
        kvfm = p_kv.tile([P, 12, NLOC], BF16, tag="kvfm")
        nc.sync.dma_start(out=kvfm[:], in_=cc_out.rearrange("(a p) n -> p a n", p=P))
        kvtm = p_kv.tile([P, 16, KVLR], BF16, tag="kvtm")
        with tc.tile_pool(name="pstp", bufs=4, space="PSUM") as ps_tp:
            for blk in range(4):
                for dsi in range(2):
                    for q4 in range(4):
                        tp = ps_tp.tile([P, P], BF16, tag="tp")
                        nc.tensor.transpose(tp[:], kvfm[:, 3 * blk + dsi, q4 * 128:(q4 + 1) * 128],
                                            identb_sb[:])
                        nc.vector.tensor_copy(
                            out=kvtm[:, blk * 4 + q4, dsi * 128:(dsi + 1) * 128], in_=tp[:])

        # ================= attention =================
        p_mask = g_mask.enter_context(tc.tile_pool(name="pmask", bufs=1, side="right"))
        m1_sb = p_mask.tile([P, 8, NLOC], BF16, tag="m1")
        nc.sync.dma_start(out=m1_sb[:], in_=mask1.rearrange("a p n -> p a n"))
        m2_sb = p_mask.tile([P, 8, 256], BF16, tag="m2")
        nc.sync.dma_start(out=m2_sb[:], in_=mask2.rearrange("a p n -> p a n"))
        wv_sb = p_y.tile([P, 2, H * VD], BF16, tag="wv")
        nc.sync.dma_start(out=wv_sb[:], in_=wv_w[:])
        yT = p_y.tile([P, 8, NLOC], BF16, tag="yT")

        with tc.tile_pool(name="psst", bufs=2, space="PSUM") as ps_st, \
             tc.tile_pool(name="psol", bufs=1, space="PSUM") as ps_ol, \
             tc.tile_pool(name="psden", bufs=1, space="PSUM") as ps_den, \
             tc.tile_pool(name="patt", bufs=2) as p_att:
            for h in range(H):
                # olc{0,1}: PV accumulators for latent chunks, all 512 queries.
                # den/ol accumulation groups span columns 0:512 for k-tiles
                # 0..7 and 256:512 for 8..15 (stop flags are sim-only).
                olc0 = ps_ol.tile([P, NLOC], F32, tag="olc0")
                olc1 = ps_ol.tile([P, NLOC], F32, tag="olc1")
                denAB = ps_den.tile([1, NLOC], F32, tag="den")
                for kt in range(16):
                    blk, ktl = _kmap(kt)
                    slab1 = kt < 8
                    w = NLOC if slab1 else 256
                    qof = 0 if slab1 else 256
                    kc = slice(ktl * 128, (ktl + 1) * 128)
                    st = ps_st.tile([P, NLOC], F32, tag="st")
                    nc.tensor.matmul(st[:, :w], kvfm[:, 3 * blk + 0, kc],
                                     qabs[:, 2 * h + 0, qof:NLOC], start=True, stop=False)
                    nc.tensor.matmul(st[:, :w], kvfm[:, 3 * blk + 1, kc],
                                     qabs[:, 2 * h + 1, qof:NLOC], start=False, stop=False)
                    nc.tensor.matmul(st[:, :w], kvfm[0:ROPE, 3 * blk + 2, kc],
                                     qrope[:, h, qof:NLOC], start=False, stop=True)
                    msb = m1_sb[:, kt, :] if slab1 else m2_sb[:, kt - 8, :]
                    nc.vector.tensor_tensor(out=st[:, :w], in0=st[:, :w], in1=msb, op=AL.add)
                    pt = p_att.tile([P, NLOC], BF16, tag="P")
                    nc.scalar.activation(out=pt[:, :w], in_=st[:, :w], func=AF.Exp)
                    tmi = blk * 4 + ktl
                    if slab1:
                        nc.tensor.matmul(denAB[:, :], onesbf_sb[:], pt[:, :],
                                         start=(kt == 0), stop=False)
                        nc.tensor.matmul(olc0[:, :], kvtm[:, tmi, 0:128], pt[:, :],
                                         start=(kt == 0), stop=False)
                        nc.tensor.matmul(olc1[:, :], kvtm[:, tmi, 128:256], pt[:, :],
                                         start=(kt == 0), stop=False)
                    else:
                        nc.tensor.matmul(denAB[:, 256:NLOC], onesbf_sb[:], pt[:, 0:256],
                                         start=False, stop=(kt == 15))
                        nc.tensor.matmul(olc0[:, 256:NLOC], kvtm[:, tmi, 0:128], pt[:, 0:256],
                                         start=False, stop=(kt == 15))
                        nc.tensor.matmul(olc1[:, 256:NLOC], kvtm[:, tmi, 128:256], pt[:, 0:256],
                                         start=False, stop=(kt == 15))
                drow = p_att.tile([1, NLOC], F32, tag="drow")
                nc.vector.tensor_copy(out=drow[:], in_=denAB[:])
                rAB = brec(ps_st, p_att, drow[:, :], NLOC, tag="st")
                olq = p_att.tile([P, 2, NLOC], BF16, tag="olq")
                nc.vector.tensor_tensor(out=olq[:, 0, :], in0=olc0[:], in1=rAB[:], op=AL.mult)
                nc.vector.tensor_tensor(out=olq[:, 1, :], in0=olc1[:], in1=rAB[:], op=AL.mult)
                yp = ps_st.tile([P, NLOC], F32, tag="st")
                nc.tensor.matmul(yp[:], wv_sb[:, 0, h * VD:(h + 1) * VD], olq[:, 0, :],
                                 start=True, stop=False)
                nc.tensor.matmul(yp[:], wv_sb[:, 1, h * VD:(h + 1) * VD], olq[:, 1, :],
                                 start=False, stop=True)
                nc.vector.tensor_copy(out=yT[:, h, :], in_=yp[:])

        g_mask.close()
        g_kv.close()
        g_q.close()

        # ================= c_proj + residual, rms2, gate, MoE =================
        with tc.tile_pool(name="pwmlp", bufs=3) as p_wmlp, \
             tc.tile_pool(name="px2", bufs=1) as p_x2, \
             tc.tile_pool(name="pspx", bufs=1) as p_spx, \
             tc.tile_pool(name="pgat", bufs=2) as p_gat, \
             tc.tile_pool(name="psmm", bufs=4, space="PSUM") as ps_mm, \
             tc.tile_pool(name="psrow2", bufs=1, space="PSUM") as ps_row2, \
             tc.tile_pool(name="psbcm", bufs=1, space="PSUM") as ps_bcm, \
             tc.tile_pool(name="pstp2", bufs=2, space="PSUM") as ps_tp2:

            x2 = p_x2.tile([P, 8, NLOC], F32, tag="x2")
            # preload x2 with full-precision x, then add c_proj output in place
            nc.sync.dma_start(out=x2[:], in_=xT.rearrange("(a p) n -> p a n", p=P))
            for half in range(2):
                cw_sb = p_wmlp.tile([P, 8, 512], BF16, tag="w16")
                nc.sync.dma_start(out=cw_sb[:], in_=cproj_w[:, :, half * 512:(half + 1) * 512])
                for ft in range(4):
                    gft = half * 4 + ft
                    op = ps_mm.tile([P, NLOC], F32, tag="mm")
                    for ds in range(8):
                        nc.tensor.matmul(op[:], cw_sb[:, ds, ft * 128:(ft + 1) * 128],
                                         yT[:, ds, :], start=(ds == 0), stop=(ds == 7))
                    nc.vector.tensor_add(out=x2[:, gft, :], in0=op[:], in1=x2[:, gft, :])
            g_y.close()
            g_x.close()

            # rms2 (exact fp32 sumsq: feeds the gate)
            ss2 = ps_row2.tile([1, NLOC], F32, tag="row2")
            for ds in range(8):
                xsq = p_sc.tile([P, NLOC], F32, tag="xsq32")
                nc.vector.tensor_mul(out=xsq[:], in0=x2[:, ds, :], in1=x2[:, ds, :])
                nc.tensor.matmul(ss2[:], ones32_sb[:], xsq[:],
                                 start=(ds == 0), stop=(ds == 7))
            nc.scalar.activation(out=rows_sb[:, 1, :], in_=ss2[:],
                                 func=AF.Sqrt, bias=eps1[:], scale=1.0 / D)
            h2 = p_x2.tile([P, 8, NLOC], BF16, tag="h2")
            s2b = brec(ps_bcm, p_sc, rows_sb[:, 1, :])
            for ds in range(8):
                nc.vector.tensor_tensor(out=h2[:, ds, :], in0=x2[:, ds, :],
                                        in1=s2b[:], op=AL.mult)
            # token-major copy of h2 for the per-expert dispatch gathers
            # (column NLOC is a zeroed dead slot for capacity padding)
            h2tm = p_x2.tile([P, NLOC + 1, 8], BF16, tag="h2tm")
            nc.vector.memset(h2tm[:, NLOC, :], 0.0)
            for ds in range(8):
                nc.vector.tensor_copy(out=h2tm[:, :NLOC, ds], in_=h2[:, ds, :])

            # gate: exact fp32 logits -> softmax -> top2 combine weights
            gw_sb = p_const.tile([P, 8, E], F32, tag="gw")
            nc.sync.dma_start(out=gw_sb[:], in_=gate_w[:])
            gp = ps_mm.tile([P, NLOC], F32, tag="mm")
            for ds in range(8):
                nc.tensor.matmul(gp[:E], gw_sb[:, ds, :], x2[:, ds, :],
                                 start=(ds == 0), stop=(ds == 7))
            g_sb = p_sc.tile([E, NLOC], F32, tag="gsb")
            nc.vector.tensor_copy(out=g_sb[:], in_=gp[:E])
            cwT = p_const.tile([E, NLOC], BF16, tag="cwT")
            valblk = p_spx.tile([P, E, 4], F32, tag="valblk")
            for q4 in range(4):
                tp = ps_tp2.tile([P, P], F32, tag="tp2")
                nc.tensor.transpose(tp[:, 0:E], g_sb[:, q4 * 128:(q4 + 1) * 128],
                                    identf[0:E, 0:E])
                gt = p_sc.tile([P, E], F32, tag="gt")
                nc.vector.tensor_copy(out=gt[:], in_=tp[:, 0:E])
                s2tp = ps_tp2.tile([P, P], F32, tag="tp2")
                nc.tensor.transpose(s2tp[:, 0:1], rows_sb[:, 1, q4 * 128:(q4 + 1) * 128],
                                    identf[0:1, 0:1])
                s2c = p_sc.tile([P, 1], F32, tag="s2c")
                nc.vector.reciprocal(out=s2c[:], in_=s2tp[:, 0:1])
                nc.vector.tensor_scalar_mul(out=gt[:], in0=gt[:], scalar1=s2c[:])
                mx = p_sc.tile([P, 4], F32, tag="mx")
                nc.vector.tensor_reduce(out=mx[:, 0:1], in_=gt[:], axis=mybir.AxisListType.X,
                                        op=AL.max)
                nc.vector.tensor_scalar_mul(out=mx[:, 1:2], in0=mx[:, 0:1], scalar1=-1.0)
                e8 = p_sc.tile([P, E], F32, tag="e8")
                nc.scalar.activation(out=e8[:], in_=gt[:], func=AF.Exp,
                                     bias=mx[:, 1:2], accum_out=mx[:, 2:3])
                nc.vector.reciprocal(out=mx[:, 3:4], in_=mx[:, 2:3])
                srt = p_sc.tile([P, E], F32, tag="srt")
                nc.vector.max(out=srt[:], in_=e8[:])
                cwq = p_sc.tile([P, E], F32, tag="cwq")
                nc.vector.tensor_scalar(out=cwq[:], in0=e8[:], scalar1=srt[:, 1:2],
                                        scalar2=None, op0=AL.is_ge)
                # candidate values for compaction: token id if selected else -1
                tkid = p_sc.tile([P, 1], F32, tag="tkid")
                nc.vector.tensor_scalar(out=tkid[:], in0=iota_sb[:],
                                        scalar1=float(128 * q4), scalar2=None, op0=AL.add)
                nc.vector.tensor_scalar_mul(out=valblk[:, :, q4], in0=cwq[:],
                                            scalar1=tkid[:, 0:1])
                nc.vector.tensor_add(out=valblk[:, :, q4], in0=valblk[:, :, q4], in1=cwq[:])
                nc.vector.tensor_scalar(out=valblk[:, :, q4], in0=valblk[:, :, q4],
                                        scalar1=-1.0, scalar2=None, op0=AL.add)
                # combine weights (zero for unselected)
                nc.vector.tensor_mul(out=cwq[:], in0=cwq[:], in1=e8[:])
                nc.vector.tensor_scalar_mul(out=cwq[:], in0=cwq[:], scalar1=mx[:, 3:4])
                tp2 = ps_tp2.tile([P, P], F32, tag="tp2")
                nc.tensor.transpose(tp2[0:E, 0:P], cwq[:], identf[:])
                nc.vector.tensor_copy(out=cwT[:, q4 * 128:(q4 + 1) * 128], in_=tp2[0:E, 0:P])

            # compact per-expert token lists: fold candidates to the wrap-16
            # grid via a DRAM round-trip (DVE partition windows must be
            # 32-aligned; DMA partition rearranges are unrestricted),
            # stream-compact with sparse_gather, then replicate the indices
            # across the eight 16-partition groups with a selector matmul.
            rep16_sb = p_const.tile([16, P], F32, tag="rep16")
            nc.sync.dma_start(out=rep16_sb[:], in_=rep16[:])
            nc.sync.dma_start(out=vald[:], in_=valblk[:])
            # 32 real candidate columns + CI pad columns holding the dead
            # token id 512: sparse_gather's output beyond the true count is
            # then always 512 (pads scan after the real columns), never the
            # arbitrary junk the ucode leaves past num_found.
            sel16 = p_spx.tile([16, E, 11, 4], F32, tag="sel16")
            nc.vector.memset(sel16[:], float(NLOC))
            vals = vald.rearrange("(k s) e b -> s e k b", s=16)
            for e in range(E):
                nc.sync.dma_start(out=sel16[:, e, 0:8, :], in_=vals[:, e, :, :])
            cidx_all = p_spx.tile([16, E, CI], F32, tag="cidx")
            cnt = p_spx.tile([1, E], U32, tag="cnt")
            for e in range(E):
                nc.gpsimd.sparse_gather(out=cidx_all[:, e, :], in_=sel16[:, e, :, :],
                                        num_found=cnt[:, e:e + 1])
            idxp = ps_tp2.tile([P, E * CI], F32, tag="tp2")
            nc.tensor.matmul(idxp[:], rep16_sb[:], cidx_all[:, :, :],
                             start=True, stop=True)
            idx16 = p_spx.tile([P, E, CI], I16, tag="idx16")
            nc.vector.tensor_copy(out=idx16[:], in_=idxp[:])

            # shared expert (weights streamed in halves, tag w16)
            macc = p_x2.tile([P, 8, NLOC], F32, tag="macc")
            hsh = p_x2.tile([P, 8, NLOC], BF16, tag="hsh")
            for half in range(2):
                w1_sb = p_wmlp.tile([P, 8, 512], BF16, tag="w16")
                nc.sync.dma_start(out=w1_sb[:], in_=shw1[:, :, half * 512:(half + 1) * 512])
                w3_sb = p_wmlp.tile([P, 8, 512], BF16, tag="w16")
                nc.sync.dma_start(out=w3_sb[:], in_=shw3[:, :, half * 512:(half + 1) * 512])
                for ft in range(4):
                    gft = half * 4 + ft
                    g1 = ps_mm.tile([P, NLOC], F32, tag="mm")
                    g3 = ps_mm.tile([P, NLOC], F32, tag="mm")
                    for ds in range(8):
                        nc.tensor.matmul(g1[:], w1_sb[:, ds, ft * 128:(ft + 1) * 128],
                                         h2[:, ds, :], start=(ds == 0), stop=(ds == 7))
                    for ds in range(8):
                        nc.tensor.matmul(g3[:], w3_sb[:, ds, ft * 128:(ft + 1) * 128],
                                         h2[:, ds, :], start=(ds == 0), stop=(ds == 7))
                    sl = p_sc.tile([P, NLOC], F32, tag="silu")
                    nc.scalar.activation(out=sl[:], in_=g1[:], func=AF.Silu)
                    nc.vector.tensor_mul(out=hsh[:, gft, :], in0=sl[:], in1=g3[:])
            for half in range(2):
                w2_sb = p_wmlp.tile([P, 8, 512], BF16, tag="w16")
                nc.sync.dma_start(out=w2_sb[:], in_=shw2[:, :, half * 512:(half + 1) * 512])
                for ft in range(4):
                    gft = half * 4 + ft
                    op = ps_mm.tile([P, NLOC], F32, tag="mm")
                    for ds in range(8):
                        nc.tensor.matmul(op[:], w2_sb[:, ds, ft * 128:(ft + 1) * 128],
                                         hsh[:, ds, :], start=(ds == 0), stop=(ds == 7))
                    nc.vector.tensor_add(out=macc[:, gft, :], in0=op[:], in1=x2[:, gft, :])

            # routed experts: sparse over compacted token lists (capacity C)
            ymoe = p_x2.tile([P, NLOC + 1, 8], BF16, tag="ymoe")
            nc.vector.memset(ymoe[:], 0.0)
            for e in range(E):
                e1_sb = p_wmlp.tile([P, 8, INTER], BF16, tag="w16")
                nc.sync.dma_start(out=e1_sb[:], in_=ew1[e])
                e3_sb = p_wmlp.tile([P, 8, INTER], BF16, tag="w16")
                nc.sync.dma_start(out=e3_sb[:], in_=ew3[e])
                # combine-weight row -> broadcast -> compact (cw of junk-pad
                # slots is harmless: ap_gather maps negative idxs to token 0,
                # scatter_add ignores negative idxs)
                cwp = ps_bcm.tile([P, NLOC], F32, tag="bc")
                nc.tensor.matmul(cwp[:], sel8_sb[:, e * P:(e + 1) * P], cwT[:],
                                 start=True, stop=True)
                cwb = p_sc.tile([P, NLOC + 1, 1], F32, tag="bcsb")
                nc.vector.memset(cwb[:, NLOC, :], 0.0)
                nc.vector.tensor_copy(out=cwb[:, :NLOC, 0], in_=cwp[:])
                cwc = p_gat.tile([P, C, 1], F32, tag="cwc")
                nc.gpsimd.ap_gather(out_ap=cwc[:], in_ap=cwb[:], idxs_ap=idx16[:, e, :],
                                    channels=P, num_elems=NLOC + 1, d=1, num_idxs=C)
                h2c = p_gat.tile([P, C, 8], BF16, tag="h2c")
                nc.gpsimd.ap_gather(out_ap=h2c[:], in_ap=h2tm[:], idxs_ap=idx16[:, e, :],
                                    channels=P, num_elems=NLOC + 1, d=8, num_idxs=C)
                hec = p_gat.tile([P, 4, C], BF16, tag="hec")
                for ft in range(4):
                    g1 = ps_mm.tile([P, NLOC], F32, tag="mm")
                    g3 = ps_mm.tile([P, NLOC], F32, tag="mm")
                    for ds in range(8):
                        nc.tensor.matmul(g1[:, :C], e1_sb[:, ds, ft * 128:(ft + 1) * 128],
                                         h2c[:, :, ds], start=(ds == 0), stop=(ds == 7))
                    for ds in range(8):
                        nc.tensor.matmul(g3[:, :C], e3_sb[:, ds, ft * 128:(ft + 1) * 128],
                                         h2c[:, :, ds], start=(ds == 0), stop=(ds == 7))
                    sl = p_sc.tile([P, NLOC], F32, tag="silu")
                    nc.scalar.activation(out=sl[:, :C], in_=g1[:, :C], func=AF.Silu)
                    nc.vector.tensor_mul(out=hec[:, ft, :], in0=sl[:, :C], in1=g3[:, :C])
                e2_sb = p_wmlp.tile([P, 4, D], BF16, tag="w16")
                nc.sync.dma_start(out=e2_sb[:], in_=ew2[e])
                oute = p_gat.tile([P, C, 8], BF16, tag="oute")
                for ft in range(8):
                    op = ps_mm.tile([P, NLOC], F32, tag="mm")
                    for ds in range(4):
                        nc.tensor.matmul(op[:, :C], e2_sb[:, ds, ft * 128:(ft + 1) * 128],
                                         hec[:, ds, :], start=(ds == 0), stop=(ds == 3))
                    nc.vector.tensor_tensor(out=oute[:, :, ft], in0=op[:, :C],
                                            in1=cwc[:, :, 0], op=AL.mult)
                nc.gpsimd.scatter_add(in_ap=ymoe[:], idxs_ap=idx16[:, e, :], add_ap=oute[:],
                                      channels=P, num_elems=NLOC + 1, d=8, num_idxs=C)
            for ds in range(8):
                nc.vector.tensor_tensor(out=macc[:, ds, :], in0=macc[:, ds, :],
                                        in1=ymoe[:, :NLOC, ds], op=AL.add)

            nc.sync.dma_start(out=out_xT.rearrange("(a p) n -> p a n", p=P), in_=macc[:])

    nc.finalize()
    return nc


# ============================ host side ============================

_CACHE = {}


def _prep_shared(inputs):
    perm = _rope_perm()
    latent_w = inputs["latent_w"] * inputs["rmsn1_w"][:, None]
    latent_w = latent_w.copy()
    latent_w[:, QLR + KVLR:] = latent_w[:, QLR + KVLR:][:, perm]
    q_up = inputs["q_up_w"] * inputs["q_norm_w"][:, None]
    q_up = q_up.copy()
    for h in range(H):
        c0 = h * QKD + NOPE
        q_up[:, c0:c0 + ROPE] = q_up[:, c0:c0 + ROPE][:, perm]
    kv_up = inputs["kv_up_w"] * inputs["kv_norm_w"][:, None]
    wv = np.concatenate([kv_up[:, h * (NOPE + VD) + NOPE:(h + 1) * (NOPE + VD)]
                         for h in range(H)], axis=1)  # [KVLR, H*VD]
    # host-absorbed q projection: q_abs = (Wq_nope @ Wkv_nope^T)^T qln
    wabs = np.concatenate(
        [q_up[:, h * QKD:h * QKD + NOPE]
         @ kv_up[:, h * (NOPE + VD):h * (NOPE + VD) + NOPE].T
         for h in range(H)], axis=1)  # [QLR, H*KVLR]
    r2 = inputs["rmsn2_w"][:, None]
    f = np.float32
    bf = ml_dtypes.bfloat16
    shared = {
        "ident": np.eye(P, dtype=f),
        "identb": np.eye(P, dtype=bf),
        "ones32": np.ones((P, 1), dtype=f),
        "onesbf": np.ones((P, 1), dtype=bf),
        "iota": np.arange(P, dtype=f).reshape(P, 1),
        "lat_w": _tile_w(latent_w.astype(f)).astype(bf),
        "q_up": _tile_w(q_up.astype(f)).astype(bf),
        "wabs": _tile_w(wabs.astype(f)).astype(bf),
        "wv_w": _tile_w(wv.astype(f)).astype(bf),
        "cproj_w": _tile_w(inputs["c_proj_w"].astype(f)).astype(bf),
        "gate_w": _tile_w((inputs["gate_w"] * r2).astype(f)),
        "shw1": _tile_w((inputs["sh_w1"] * r2).astype(f)).astype(bf),
        "shw3": _tile_w((inputs["sh_w3"] * r2).astype(f)).astype(bf),
        "shw2": _tile_w(inputs["sh_w2"].astype(f)).astype(bf),
        "ew1": np.stack([_tile_w((inputs["e_w1"][e] * r2).astype(f)) for e in range(E)]).astype(bf),
        "ew3": np.stack([_tile_w((inputs["e_w3"][e] * r2).astype(f)) for e in range(E)]).astype(bf),
        "ew2": np.stack([_tile_w(inputs["e_w2"][e].astype(f)) for e in range(E)]).astype(bf),
        "sel8": np.repeat(np.eye(E, dtype=f), P, axis=1).reshape(E, E * P).astype(bf),
        "rep16": (np.arange(P)[None, :] % 16 == np.arange(16)[:, None]).astype(f),
    }
    return shared


def _prep_core(inputs, c):
    f = np.float32
    pos = _core_positions(c)
    b = c // 4
    gidx = b * T + pos
    xflat = np.asarray(inputs["x"], dtype=f).reshape(N, D)
    xT_c = np.ascontiguousarray(xflat[gidx].T)
    cosT = np.ascontiguousarray(np.asarray(inputs["freqs_cos"], f)[pos].T)
    sinT = np.ascontiguousarray(np.asarray(inputs["freqs_sin"], f)[pos].T)
    k_abs = (np.arange(8)[:, None] * 128 + np.arange(P)[None, :])  # [8,128]
    m1 = np.where(k_abs[:, :, None] <= pos[None, None, :], 0.0, NEG)
    k_abs2 = ((np.arange(8, 16))[:, None] * 128 + np.arange(P)[None, :])
    m2 = np.where(k_abs2[:, :, None] <= pos[None, None, 256:], 0.0, NEG)
    return {
        "xT": xT_c, "xTbf": xT_c.astype(ml_dtypes.bfloat16),
        "cosT": cosT, "sinT": sinT,
        "mask1": m1.astype(ml_dtypes.bfloat16),
        "mask2": m2.astype(ml_dtypes.bfloat16),
    }, gidx


def run(inputs, trace=False, **kw):
    if "nc" not in _CACHE:
        _CACHE["nc"] = build()
    nc = _CACHE["nc"]
    shared = _prep_shared({k: np.asarray(v) for k, v in inputs.items()
                           if k not in ("x", "freqs_cos", "freqs_sin")}
                          | {k: np.asarray(inputs[k]) for k in ("x", "freqs_cos", "freqs_sin")})
    in_maps = []
    gidxs = []
    for c in range(NCORES):
        m, gidx = _prep_core(inputs, c)
        m.update(shared)
        in_maps.append(m)
        gidxs.append(gidx)
    res = run_bass_kernel_spmd(nc, in_maps, core_ids=list(range(NCORES)),
                               trace=trace, **kw)
    full = np.empty((N, D), dtype=np.float32)
    for c in range(NCORES):
        full[gidxs[c]] = res.results[c]["out_xT"].T
    return full.reshape(B, T, D), res


def kernel(**inputs):
    out, _ = run(inputs)
    return out
